# revision 1
# baseline (speedup 1.0000x reference)
"""LocalCorrelation (13x13 cost volume) Trainium2 kernel.

Full inputs z_t, z_t1: [8, 256, 128, 128] f32 -> out [8, 169, 128, 128] f32.
out[b, 13*di+dj, h, w] = sum_c z_t[b,c,h,w] * pad(z_t1)[b,c,h+di,w+dj] / 16

Sharding: data-parallel over batch, 1 batch element per NeuronCore (8 cores).

Per-core algorithm (SPMD, identical program):
  - Load z_t (scaled by 1/16) and zero-padded z_t1 into SBUF as bf16,
    channel dim on partitions (2 chunks of 128).
  - For each 8x16 output-pixel block: TensorE "block gram" matmuls
    stationary = z_t block [c,128 pixels], streaming = padded z_t1
    20x28 window [c,560] -> PSUM f32 (accumulated over 2 c-chunks).
  - PSUM -> SBUF bf16, dense DMA to DRAM scratch.
  - Shear-gather DMAs (per di) read the 13x13 tap band back into
    [di*8+dh, (w, dj)] layout -- the per-pixel diagonal offset is
    absorbed by flat DRAM addressing.
  - On-chip strided copy transposes (w,dj)->(dj,w) and casts to f32.
  - Output DMA writes [tap][h][w] with 512B runs.
"""

import numpy as np

C = 256
H = W = 128
KS = 13
KK = 169
RAD = 6
HP = WP = 140  # padded spatial
SA = 8  # block rows (stripe height)
SB = 16  # block cols
NWB = W // SB  # 8 w-blocks per stripe
NST = H // SA  # 16 stripes
WINP = SA + 2 * RAD  # 20 streamed rows per window
WINQ = SB + 2 * RAD  # 28 streamed cols per window
WIN = WINP * WINQ  # 560

_cache = {}


def _build():
    import concourse.bass as bass
    import concourse.mybir as mybir
    import concourse.tile as tile
    from concourse import bacc

    f32 = mybir.dt.float32
    bf16 = mybir.dt.bfloat16

    nc = bacc.Bacc("TRN2", target_bir_lowering=False, debug=False)
    zt_d = nc.dram_tensor("z_t", [C, H, W], f32, kind="ExternalInput")
    z1_d = nc.dram_tensor("z_t1", [C, H, W], f32, kind="ExternalInput")
    out_d = nc.dram_tensor("out", [KK, H, W], f32, kind="ExternalOutput")

    with tile.TileContext(nc) as tc:
        with tc.tile_pool(name="persist", bufs=1) as pp:
            ZT = [pp.tile([128, H * W], bf16, tag=f"zt{k}", name=f"zt{k}") for k in range(2)]
            Z1P = [pp.tile([128, HP * WP], bf16, tag=f"z1p{k}", name=f"z1p{k}") for k in range(2)]

            # ---- input load: cast f32->bf16 via SWDGE DMA ----
            # ZT is stored BLOCK-MAJOR: free index = ((si*8 + wb)*8 + dh)*16 + dw
            # so each 8x16 block's 128 pixels are contiguous (matmul stationary
            # operand requires a single free dim).
            for k in range(2):
                nc.vector.memset(Z1P[k][:, :], 0.0)

            with tc.tile_pool(name="ld", bufs=2) as ldp:
                for k in range(2):
                    for s in range(4):  # 32-row slabs
                        z1u = ldp.tile([128, 32 * W], bf16, tag="z1u", name="z1u")
                        src = z1_d.ap()[k * 128:(k + 1) * 128, s * 32:(s + 1) * 32, :]
                        nc.gpsimd.dma_start(
                            z1u.rearrange("c (h w) -> c h w", h=32), src)
                        dst = Z1P[k].rearrange("c (h w) -> c h w", h=HP)[
                            :, RAD + s * 32: RAD + (s + 1) * 32, RAD: RAD + W]
                        nc.vector.tensor_copy(dst, z1u.rearrange("c (h w) -> c h w", h=32))
                for k in range(2):
                    for s in range(4):  # 32-row slabs -> 4 stripes each
                        ztu = ldp.tile([128, 32 * W], bf16, tag="ztu", name="ztu")
                        src = zt_d.ap()[k * 128:(k + 1) * 128, s * 32:(s + 1) * 32, :]
                        nc.gpsimd.dma_start(
                            ztu.rearrange("c (h w) -> c h w", h=32), src)
                        for sl in range(4):
                            si_g = s * 4 + sl
                            srcv = ztu.rearrange(
                                "c (h wb dw) -> c wb h dw", h=32, wb=NWB)[
                                :, :, sl * SA:(sl + 1) * SA, :]
                            dstv = ZT[k][:, si_g * 1024:(si_g + 1) * 1024].rearrange(
                                "c (wb dh dw) -> c wb dh dw", wb=NWB, dh=SA)
                            nc.vector.tensor_copy(dstv, srcv)
            for k in range(2):
                nc.vector.tensor_scalar_mul(ZT[k][:, :], ZT[k][:, :], 1.0 / 16.0)

            # ---- main loop ----
            with (
                tc.tile_pool(name="xbp", bufs=2) as xbp,
                tc.tile_pool(name="o2p", bufs=2) as o2p,
                tc.tile_pool(name="o3p", bufs=2) as o3p,
                tc.tile_pool(name="psp", bufs=2, space="PSUM") as psp,
                tc.tile_pool(name="scrp", bufs=2, space="DRAM") as scrp,
            ):
                for si in range(NST):
                    h0 = si * SA
                    scr = scrp.tile([NWB, 128, WIN], bf16, tag="scr", name="scr")
                    xb = xbp.tile([128, NWB * WIN], bf16, tag="xb", name="xb")
                    for wb in range(NWB):
                        w0 = wb * SB
                        ps = [psp.tile([128, 280], f32, tag=f"ps{i}", name=f"ps{i}")
                              for i in range(2)]
                        for k in range(2):
                            blk = si * NWB + wb
                            lhsT = ZT[k][:, blk * 128:(blk + 1) * 128]
                            for half in range(2):
                                rhs = Z1P[k].rearrange("c (h w) -> c h w", h=HP)[
                                    :, h0 + 10 * half: h0 + 10 * (half + 1),
                                    w0:w0 + WINQ]
                                nc.tensor.matmul(ps[half][:, :], lhsT, rhs,
                                                 start=(k == 0), stop=(k == 1))
                        for half in range(2):
                            dst = xb[:, wb * WIN + half * 280: wb * WIN + (half + 1) * 280]
                            if wb % 2 == 0:
                                nc.scalar.copy(dst, ps[half][:, :])
                            else:
                                nc.vector.tensor_copy(dst, ps[half][:, :])

                    # dense scratch write (1120B runs per (m, wb))
                    scr_w = bass.AP(scr.tensor, 0, [[WIN, 128], [128 * WIN, NWB], [1, WIN]])
                    nc.sync.dma_start(scr_w, xb.rearrange("p (wb s) -> p wb s", wb=NWB))

                    # shear-gather: per (di, wb), absorb diagonal in DRAM strides
                    # (DMA APs are limited to 3 dims)
                    o2 = o2p.tile([104, 128 * KS], bf16, tag="o2", name="o2")
                    for di in range(KS):
                        for wb in range(NWB):
                            src = bass.AP(scr.tensor, di * WINQ + wb * 128 * WIN,
                                          [[SB * WIN + WINQ, SA],
                                           [WIN + 1, SB],
                                           [1, KS]])
                            dst = o2[di * SA:(di + 1) * SA,
                                     wb * SB * KS:(wb + 1) * SB * KS].rearrange(
                                "p (dw dj) -> p dw dj", dw=SB)
                            nc.sync.dma_start(dst, src)

                    # (w, dj) -> (dj, w) transpose + cast to f32
                    o3 = o3p.tile([104, KS * W], f32, tag="o3", name="o3")
                    src_t = o2.rearrange("p (w dj) -> p dj w", dj=KS)
                    dst_t = o3.rearrange("p (dj w) -> p dj w", dj=KS)
                    if si % 2 == 0:
                        nc.vector.tensor_copy(dst_t, src_t)
                    else:
                        nc.scalar.copy(dst_t, src_t)

                    # final output write: 512B runs
                    for di in range(KS):
                        srcw = o3[di * SA:(di + 1) * SA, :].rearrange(
                            "p (dj w) -> p dj w", dj=KS)
                        dstw = bass.AP(out_d, di * KS * H * W + h0 * W,
                                       [[W, SA], [H * W, KS], [1, W]])
                        nc.sync.dma_start(dstw, srcw)

    nc.compile()
    return nc


def _get_nc():
    if "nc" not in _cache:
        _cache["nc"] = _build()
    return _cache["nc"]


def kernel(z_t: np.ndarray, z_t1: np.ndarray) -> np.ndarray:
    from concourse.bass_utils import run_bass_kernel_spmd

    nc = _get_nc()
    z_t = np.ascontiguousarray(z_t, dtype=np.float32)
    z_t1 = np.ascontiguousarray(z_t1, dtype=np.float32)
    B = z_t.shape[0]
    in_maps = [{"z_t": z_t[i], "z_t1": z_t1[i]} for i in range(B)]
    res = run_bass_kernel_spmd(nc, in_maps, core_ids=list(range(B)))
    return np.stack([res.results[i]["out"] for i in range(B)], axis=0)



# revision 5
# speedup vs baseline: 3.0687x; 3.0687x over previous
"""LocalCorrelation (13x13 cost volume) Trainium2 kernel.

Full inputs z_t, z_t1: [8, 256, 128, 128] f32 -> out [8, 169, 128, 128] f32.
out[b, 13*di+dj, h, w] = sum_c z_t[b,c,h,w] * pad(z_t1)[b,c,h+di,w+dj] / 16

Sharding: data-parallel over batch, 1 batch element per NeuronCore (8 cores).

Per-core algorithm (SPMD, identical program):
  - Load z_t (scaled by 1/16) and zero-padded z_t1 into SBUF as bf16,
    channel dim on partitions (2 chunks of 128).
  - For each 8x16 output-pixel block: TensorE "block gram" matmuls
    stationary = z_t block [c,128 pixels], streaming = padded z_t1
    20x28 window -> PSUM f32 (accumulated over 2 c-chunks).
  - PSUM -> SBUF bf16 with 32-elem window-row pitch.
  - Sheared scratch write: DRAM addr = w*S_W + h*S_H + (p - dh)*32 + q.
    The per-pixel vertical shear is absorbed into the write AP's dh
    partition stride (864-32), so a later read at slot di*32 lands on
    window row p = dh + di for every pixel simultaneously.
  - Tap gather: ONE 3-dim DMA per (h-half, di) reads [64 h, 128 w, 28 q]
    with 56B runs -- 26 gather instructions total (vs 1664 before).
  - DVE de-shear: the residual horizontal shear q = (w mod 16) + dj is a
    free-dim diagonal, extracted by a strided copy (+ f32 cast).
  - Output write: one DMA per (h-half, di), 512B runs.
"""

import numpy as np

C = 256
H = W = 128
KS = 13
KK = 169
RAD = 6
HP = WP = 140  # padded spatial
SA = 8  # block rows (stripe height)
SB = 16  # block cols
NWB = W // SB  # 8 w-blocks per stripe
NST = H // SA  # 16 stripes
WINP = SA + 2 * RAD  # 20 streamed rows per window
WINQ = SB + 2 * RAD  # 28 streamed cols per window
QP = 32  # padded window-row pitch in xb/scratch
PQ = WINP * QP  # 640 elems per pixel in xb
SCR_H = 27 * QP  # 864: scratch stride per h (27 slots: p-dh in [-7,20))
SCR_W = 64 * SCR_H  # 55296: scratch stride per w (per h-half)
HH = H // 2  # 64 rows per h-half

_cache = {}


def _build():
    import concourse.bass as bass
    import concourse.mybir as mybir
    import concourse.tile as tile
    from concourse import bacc

    f32 = mybir.dt.float32
    bf16 = mybir.dt.bfloat16

    nc = bacc.Bacc("TRN2", target_bir_lowering=False, debug=False)
    zt_d = nc.dram_tensor("z_t", [C, H, W], f32, kind="ExternalInput")
    z1_d = nc.dram_tensor("z_t1", [C, H, W], f32, kind="ExternalInput")
    out_d = nc.dram_tensor("out", [KK, H, W], f32, kind="ExternalOutput")

    with tile.TileContext(nc) as tc:
        with tc.tile_pool(name="persist", bufs=1) as pp, \
             tc.tile_pool(name="scrp", bufs=1, space="DRAM") as scrp:
            scr_t = [scrp.tile([W, SCR_W], bf16, tag=f"scr{i}", name=f"scr{i}")
                     for i in range(2)]
            scr_d = [t.tensor for t in scr_t]
            ZT = [pp.tile([128, H * W], bf16, tag=f"zt{k}", name=f"zt{k}") for k in range(2)]
            Z1P = [pp.tile([128, HP * WP], bf16, tag=f"z1p{k}", name=f"z1p{k}") for k in range(2)]

            # ---- input load: cast f32->bf16 via SWDGE DMA ----
            # ZT is stored BLOCK-MAJOR: free index = ((si*8 + wb)*8 + dh)*16 + dw
            for k in range(2):
                nc.vector.memset(Z1P[k][:, :], 0.0)

            with tc.tile_pool(name="ld", bufs=2) as ldp:
                for k in range(2):
                    for s in range(4):  # 32-row slabs
                        z1u = ldp.tile([128, 32 * W], bf16, tag="z1u", name="z1u")
                        src = z1_d.ap()[k * 128:(k + 1) * 128, s * 32:(s + 1) * 32, :]
                        nc.gpsimd.dma_start(
                            z1u.rearrange("c (h w) -> c h w", h=32), src)
                        dst = Z1P[k].rearrange("c (h w) -> c h w", h=HP)[
                            :, RAD + s * 32: RAD + (s + 1) * 32, RAD: RAD + W]
                        nc.vector.tensor_copy(dst, z1u.rearrange("c (h w) -> c h w", h=32))
                for k in range(2):
                    for s in range(4):  # 32-row slabs -> 4 stripes each
                        ztu = ldp.tile([128, 32 * W], bf16, tag="ztu", name="ztu")
                        src = zt_d.ap()[k * 128:(k + 1) * 128, s * 32:(s + 1) * 32, :]
                        nc.gpsimd.dma_start(
                            ztu.rearrange("c (h w) -> c h w", h=32), src)
                        for sl in range(4):
                            si_g = s * 4 + sl
                            srcv = ztu.rearrange(
                                "c (h wb dw) -> c wb h dw", h=32, wb=NWB)[
                                :, :, sl * SA:(sl + 1) * SA, :]
                            dstv = ZT[k][:, si_g * 1024:(si_g + 1) * 1024].rearrange(
                                "c (wb dh dw) -> c wb dh dw", wb=NWB, dh=SA)
                            nc.vector.tensor_copy(dstv, srcv)
            for k in range(2):
                nc.vector.tensor_scalar_mul(ZT[k][:, :], ZT[k][:, :], 1.0 / 16.0)

            # ---- main loop ----
            def emit_stripe(si, xbp, psp, dma_eng):
                """Matmuls + PSUM copies + sheared scratch writes for stripe si."""
                hh, sil = divmod(si, 8)
                h0 = si * SA
                xb = xbp.tile([128, NWB * PQ], bf16, tag="xb", name="xb")
                for wb in range(NWB):
                    w0 = wb * SB
                    ps = [psp.tile([128, 10 * WINQ], f32, tag=f"ps{i}", name=f"ps{i}")
                          for i in range(2)]
                    for k in range(2):
                        blk = si * NWB + wb
                        lhsT = ZT[k][:, blk * 128:(blk + 1) * 128]
                        for half in range(2):
                            rhs = Z1P[k].rearrange("c (h w) -> c h w", h=HP)[
                                :, h0 + 10 * half: h0 + 10 * (half + 1),
                                w0:w0 + WINQ]
                            nc.tensor.matmul(ps[half][:, :], lhsT, rhs,
                                             start=(k == 0), stop=(k == 1))
                    for half in range(2):
                        dst = xb[:, wb * PQ + half * 10 * QP:
                                 wb * PQ + (half + 1) * 10 * QP].rearrange(
                            "p (r q) -> p r q", q=QP)[:, :, :WINQ]
                        srcp = ps[half].rearrange("p (r q) -> p r q", q=WINQ)
                        if wb % 2 == 0:
                            nc.scalar.copy(dst, srcp)
                        else:
                            nc.vector.tensor_copy(dst, srcp)

                # sheared scratch write: one DMA per wb, 1280B runs.
                # dst addr = w*SCR_W + h_local*SCR_H + (p - dh + 7)*32 + q
                for wb in range(NWB):
                    off = sil * SA * SCR_H + wb * SB * SCR_W + 7 * QP
                    dst = bass.AP(scr_d[hh], off,
                                  [[SCR_H - QP, SA], [SCR_W, SB], [1, PQ]])
                    dma_eng(wb).dma_start(dst, xb[:, wb * PQ:(wb + 1) * PQ])

            def emit_tap(hh, di, o4p, o5p, dma_eng):
                """Gather + de-shear + output write for (h-half, tap-row di)."""
                o4 = o4p.tile([HH, W * WINQ], bf16, tag="o4", name="o4")
                src = bass.AP(scr_d[hh], (di + 7) * QP,
                              [[SCR_H, HH], [SCR_W, W], [1, WINQ]])
                dma_eng(0).dma_start(
                    o4.rearrange("p (w q) -> p w q", q=WINQ), src)

                # de-shear: o5[p, dj*128 + wb*16 + dw] = o4[p, wb*448 + dw*29 + dj]
                o5 = o5p.tile([HH, KS * W], f32, tag="o5", name="o5")
                base = o4[:, :]
                diag = bass.AP(base.tensor, base.offset,
                               [list(base.ap[0])] +
                               [[WINQ * SB, NWB], [WINQ + 1, SB], [1, KS]])
                dst = o5[:, :].rearrange("p (dj wb dw) -> p wb dw dj",
                                         dj=KS, wb=NWB)
                if di % 2 == 0:
                    nc.vector.tensor_copy(dst, diag)
                else:
                    nc.scalar.copy(dst, diag)

                # output write: one DMA, 512B runs
                dstw = bass.AP(out_d, di * KS * H * W + hh * HH * W,
                               [[W, HH], [H * W, KS], [1, W]])
                dma_eng(1).dma_start(
                    dstw, o5[:, :].rearrange("p (dj w) -> p dj w", dj=KS))

            def alt(i):
                return nc.sync if i % 2 == 0 else nc.scalar

            with (
                tc.tile_pool(name="xbp", bufs=2) as xbp,
                tc.tile_pool(name="o4p", bufs=2) as o4p,
                tc.tile_pool(name="o5p", bufs=2) as o5p,
                tc.tile_pool(name="psp", bufs=2, space="PSUM") as psp,
            ):
                for si in range(8):
                    emit_stripe(si, xbp, psp, alt)
                # h-half 0 taps interleaved with stripes 8..15
                tap_q = list(range(KS))
                for si in range(8, 16):
                    emit_stripe(si, xbp, psp, alt)
                    for _ in range(2):
                        if tap_q:
                            emit_tap(0, tap_q.pop(0), o4p, o5p, alt)
                for di in tap_q:
                    emit_tap(0, di, o4p, o5p, alt)
                for di in range(KS):
                    emit_tap(1, di, o4p, o5p, alt)

    nc.compile()
    return nc


def _get_nc():
    if "nc" not in _cache:
        _cache["nc"] = _build()
    return _cache["nc"]


def kernel(z_t: np.ndarray, z_t1: np.ndarray) -> np.ndarray:
    from concourse.bass_utils import run_bass_kernel_spmd

    nc = _get_nc()
    z_t = np.ascontiguousarray(z_t, dtype=np.float32)
    z_t1 = np.ascontiguousarray(z_t1, dtype=np.float32)
    B = z_t.shape[0]
    in_maps = [{"z_t": z_t[i], "z_t1": z_t1[i]} for i in range(B)]
    res = run_bass_kernel_spmd(nc, in_maps, core_ids=list(range(B)))
    return np.stack([res.results[i]["out"] for i in range(B)], axis=0)


# revision 6
# speedup vs baseline: 3.0697x; 1.0003x over previous
"""LocalCorrelation (13x13 cost volume) Trainium2 kernel.

Full inputs z_t, z_t1: [8, 256, 128, 128] f32 -> out [8, 169, 128, 128] f32.
out[b, 13*di+dj, h, w] = sum_c z_t[b,c,h,w] * pad(z_t1)[b,c,h+di,w+dj] / 16

Sharding: data-parallel over batch, 1 batch element per NeuronCore (8 cores).

Per-core algorithm (SPMD, identical program):
  - Load z_t (scaled by 1/16) and zero-padded z_t1 into SBUF as bf16,
    channel dim on partitions (2 chunks of 128).
  - For each 8x16 output-pixel block: TensorE "block gram" matmuls
    stationary = z_t block [c,128 pixels], streaming = padded z_t1
    20x28 window -> PSUM f32 (accumulated over 2 c-chunks).
  - PSUM -> SBUF bf16 with 32-elem window-row pitch.
  - Sheared scratch write: DRAM addr = w*S_W + h*S_H + (p - dh)*32 + q.
    The per-pixel vertical shear is absorbed into the write AP's dh
    partition stride (864-32), so a later read at slot di*32 lands on
    window row p = dh + di for every pixel simultaneously.
  - Tap gather: ONE 3-dim DMA per (h-half, di) reads [64 h, 128 w, 28 q]
    with 56B runs -- 26 gather instructions total (vs 1664 before).
  - DVE de-shear: the residual horizontal shear q = (w mod 16) + dj is a
    free-dim diagonal, extracted by a strided copy (+ f32 cast).
  - Output write: one DMA per (h-half, di), 512B runs.
"""

import numpy as np

C = 256
H = W = 128
KS = 13
KK = 169
RAD = 6
HP = WP = 140  # padded spatial
SA = 8  # block rows (stripe height)
SB = 16  # block cols
NWB = W // SB  # 8 w-blocks per stripe
NST = H // SA  # 16 stripes
WINP = SA + 2 * RAD  # 20 streamed rows per window
WINQ = SB + 2 * RAD  # 28 streamed cols per window
QP = 32  # padded window-row pitch in xb/scratch
PQ = WINP * QP  # 640 elems per pixel in xb
SCR_H = 27 * QP  # 864: scratch stride per h (27 slots: p-dh in [-7,20))
SCR_W = 64 * SCR_H  # 55296: scratch stride per w (per h-half)
HH = H // 2  # 64 rows per h-half

_cache = {}


def _build():
    import concourse.bass as bass
    import concourse.mybir as mybir
    import concourse.tile as tile
    from concourse import bacc

    f32 = mybir.dt.float32
    bf16 = mybir.dt.bfloat16

    nc = bacc.Bacc("TRN2", target_bir_lowering=False, debug=False)
    zt_d = nc.dram_tensor("z_t", [C, H, W], f32, kind="ExternalInput")
    z1_d = nc.dram_tensor("z_t1", [C, H, W], f32, kind="ExternalInput")
    out_d = nc.dram_tensor("out", [KK, H, W], f32, kind="ExternalOutput")

    with tile.TileContext(nc) as tc:
        with tc.tile_pool(name="persist", bufs=1) as pp, \
             tc.tile_pool(name="scrp", bufs=1, space="DRAM") as scrp:
            scr_t = [scrp.tile([W, SCR_W], bf16, tag=f"scr{i}", name=f"scr{i}")
                     for i in range(2)]
            scr_d = [t.tensor for t in scr_t]
            ZT = [pp.tile([128, H * W], bf16, tag=f"zt{k}", name=f"zt{k}") for k in range(2)]
            Z1P = [pp.tile([128, HP * WP], bf16, tag=f"z1p{k}", name=f"z1p{k}") for k in range(2)]

            # ---- input load: cast f32->bf16 via SWDGE DMA ----
            # ZT is stored BLOCK-MAJOR: free index = ((si*8 + wb)*8 + dh)*16 + dw
            for k in range(2):
                nc.vector.memset(Z1P[k][:, :], 0.0)

            with tc.tile_pool(name="ld", bufs=2) as ldp:
                for k in range(2):
                    for s in range(4):  # 32-row slabs
                        z1u = ldp.tile([128, 32 * W], bf16, tag="z1u", name="z1u")
                        src = z1_d.ap()[k * 128:(k + 1) * 128, s * 32:(s + 1) * 32, :]
                        nc.gpsimd.dma_start(
                            z1u.rearrange("c (h w) -> c h w", h=32), src)
                        dst = Z1P[k].rearrange("c (h w) -> c h w", h=HP)[
                            :, RAD + s * 32: RAD + (s + 1) * 32, RAD: RAD + W]
                        nc.vector.tensor_copy(dst, z1u.rearrange("c (h w) -> c h w", h=32))
                for k in range(2):
                    for s in range(4):  # 32-row slabs -> 4 stripes each
                        ztu = ldp.tile([128, 32 * W], bf16, tag="ztu", name="ztu")
                        src = zt_d.ap()[k * 128:(k + 1) * 128, s * 32:(s + 1) * 32, :]
                        nc.gpsimd.dma_start(
                            ztu.rearrange("c (h w) -> c h w", h=32), src)
                        for sl in range(4):
                            si_g = s * 4 + sl
                            srcv = ztu.rearrange(
                                "c (h wb dw) -> c wb h dw", h=32, wb=NWB)[
                                :, :, sl * SA:(sl + 1) * SA, :]
                            dstv = ZT[k][:, si_g * 1024:(si_g + 1) * 1024].rearrange(
                                "c (wb dh dw) -> c wb dh dw", wb=NWB, dh=SA)
                            nc.vector.tensor_copy(dstv, srcv)
            for k in range(2):
                nc.vector.tensor_scalar_mul(ZT[k][:, :], ZT[k][:, :], 1.0 / 16.0)

            # ---- main loop ----
            def emit_stripe(si, xbp, psp, dma_eng):
                """Matmuls + PSUM copies + sheared scratch writes for stripe si."""
                hh, sil = divmod(si, 8)
                h0 = si * SA
                xb = xbp.tile([128, NWB * PQ], bf16, tag="xb", name="xb")
                for wb in range(NWB):
                    w0 = wb * SB
                    ps = [psp.tile([128, 10 * WINQ], f32, tag=f"ps{i}", name=f"ps{i}")
                          for i in range(2)]
                    for k in range(2):
                        blk = si * NWB + wb
                        lhsT = ZT[k][:, blk * 128:(blk + 1) * 128]
                        for half in range(2):
                            rhs = Z1P[k].rearrange("c (h w) -> c h w", h=HP)[
                                :, h0 + 10 * half: h0 + 10 * (half + 1),
                                w0:w0 + WINQ]
                            nc.tensor.matmul(ps[half][:, :], lhsT, rhs,
                                             start=(k == 0), stop=(k == 1))
                    for half in range(2):
                        dst = xb[:, wb * PQ + half * 10 * QP:
                                 wb * PQ + (half + 1) * 10 * QP].rearrange(
                            "p (r q) -> p r q", q=QP)[:, :, :WINQ]
                        srcp = ps[half].rearrange("p (r q) -> p r q", q=WINQ)
                        if wb % 2 == 0:
                            nc.scalar.copy(dst, srcp)
                        else:
                            nc.vector.tensor_copy(dst, srcp)

                # sheared scratch write: one DMA per wb, 1280B runs.
                # dst addr = w*SCR_W + h_local*SCR_H + (p - dh + 7)*32 + q
                for wb in range(NWB):
                    off = sil * SA * SCR_H + wb * SB * SCR_W + 7 * QP
                    dst = bass.AP(scr_d[hh], off,
                                  [[SCR_H - QP, SA], [SCR_W, SB], [1, PQ]])
                    dma_eng(wb).dma_start(dst, xb[:, wb * PQ:(wb + 1) * PQ])

            def emit_tap(hh, di, o4p, o5p, dma_eng):
                """Gather + de-shear + output write for (h-half, tap-row di)."""
                o4 = o4p.tile([HH, W * WINQ], bf16, tag="o4", name="o4")
                src = bass.AP(scr_d[hh], (di + 7) * QP,
                              [[SCR_H, HH], [SCR_W, W], [1, WINQ]])
                dma_eng(0).dma_start(
                    o4.rearrange("p (w q) -> p w q", q=WINQ), src,
                    single_packet=True)

                # de-shear: o5[p, dj*128 + wb*16 + dw] = o4[p, wb*448 + dw*29 + dj]
                o5 = o5p.tile([HH, KS * W], f32, tag="o5", name="o5")
                base = o4[:, :]
                diag = bass.AP(base.tensor, base.offset,
                               [list(base.ap[0])] +
                               [[WINQ * SB, NWB], [WINQ + 1, SB], [1, KS]])
                dst = o5[:, :].rearrange("p (dj wb dw) -> p wb dw dj",
                                         dj=KS, wb=NWB)
                if di % 2 == 0:
                    nc.vector.tensor_copy(dst, diag)
                else:
                    nc.scalar.copy(dst, diag)

                # output write: one DMA, 512B runs
                dstw = bass.AP(out_d, di * KS * H * W + hh * HH * W,
                               [[W, HH], [H * W, KS], [1, W]])
                dma_eng(1).dma_start(
                    dstw, o5[:, :].rearrange("p (dj w) -> p dj w", dj=KS))

            def alt(i):
                return nc.sync if i % 2 == 0 else nc.scalar

            with (
                tc.tile_pool(name="xbp", bufs=2) as xbp,
                tc.tile_pool(name="o4p", bufs=2) as o4p,
                tc.tile_pool(name="o5p", bufs=2) as o5p,
                tc.tile_pool(name="psp", bufs=2, space="PSUM") as psp,
            ):
                for si in range(8):
                    emit_stripe(si, xbp, psp, alt)
                # h-half 0 taps interleaved with stripes 8..15
                tap_q = list(range(KS))
                for si in range(8, 16):
                    emit_stripe(si, xbp, psp, alt)
                    for _ in range(2):
                        if tap_q:
                            emit_tap(0, tap_q.pop(0), o4p, o5p, alt)
                for di in tap_q:
                    emit_tap(0, di, o4p, o5p, alt)
                for di in range(KS):
                    emit_tap(1, di, o4p, o5p, alt)

    nc.compile()
    return nc


def _get_nc():
    if "nc" not in _cache:
        _cache["nc"] = _build()
    return _cache["nc"]


def kernel(z_t: np.ndarray, z_t1: np.ndarray) -> np.ndarray:
    from concourse.bass_utils import run_bass_kernel_spmd

    nc = _get_nc()
    z_t = np.ascontiguousarray(z_t, dtype=np.float32)
    z_t1 = np.ascontiguousarray(z_t1, dtype=np.float32)
    B = z_t.shape[0]
    in_maps = [{"z_t": z_t[i], "z_t1": z_t1[i]} for i in range(B)]
    res = run_bass_kernel_spmd(nc, in_maps, core_ids=list(range(B)))
    return np.stack([res.results[i]["out"] for i in range(B)], axis=0)


# revision 9
# speedup vs baseline: 3.7201x; 1.2119x over previous
"""LocalCorrelation (13x13 cost volume) Trainium2 kernel.

Full inputs z_t, z_t1: [8, 256, 128, 128] f32 -> out [8, 169, 128, 128] f32.
out[b, 13*di+dj, h, w] = sum_c z_t[b,c,h,w] * pad(z_t1)[b,c,h+di,w+dj] / 16

Sharding: data-parallel over batch, 1 batch element per NeuronCore (8 cores).

Per-core algorithm (SPMD, identical program):
  - Slab-staged input load (f32->bf16 SWDGE cast) interleaved with compute;
    the 1/sqrt(C) scale is fused into the z_t reorder copy.
  - Per 8x16 output-pixel block: TensorE "block gram" matmuls, stationary =
    z_t block [c,128 pixels], streaming = padded z_t1 20x28 window -> PSUM
    f32 (accumulated over 2 c-chunks of 128).
  - PSUM -> SBUF bf16 with 32-elem window-row pitch.
  - Sheared scratch write: DRAM addr = w*S_W + h*S_H + (p - dh + 7)*32 + q.
    The per-pixel vertical shear is absorbed into the write AP's dh
    partition stride (S_H - 32), so window row p = dh + di for every pixel
    lands at slot di+7.
  - Band gather: per (h-half, wb) one 3-dim DMA reads slots [7,20) of every
    pixel -- 832B contiguous runs, 16 instructions, all 13 tap rows at once.
  - DVE de-shear: slot pick + horizontal diagonal q = (w mod 16) + dj via
    strided copy (+ f32 cast) into per-di output tiles.
  - Output write: one DMA per (h-half, di), 512B runs.
"""

import numpy as np

C = 256
H = W = 128
KS = 13
KK = 169
RAD = 6
HP = WP = 140  # padded spatial
SA = 8  # block rows (stripe height)
SB = 16  # block cols
NWB = W // SB  # 8 w-blocks per stripe
NST = H // SA  # 16 stripes
WINP = SA + 2 * RAD  # 20 streamed rows per window
WINQ = SB + 2 * RAD  # 28 streamed cols per window
QP = 32  # padded window-row pitch in xb/scratch
PQ = WINP * QP  # 640 elems per pixel in xb
SCR_H = 27 * QP  # 864: scratch stride per h (27 slots: p-dh+7 in [0,27))
SCR_W = 64 * SCR_H  # 55296: scratch stride per w (per h-half)
HH = H // 2  # 64 rows per h-half
BAND = KS * QP  # 416: gathered band elems per pixel (slots 7..19)

_cache = {}


def _build():
    import concourse.bass as bass
    import concourse.mybir as mybir
    import concourse.tile as tile
    from concourse import bacc

    f32 = mybir.dt.float32
    bf16 = mybir.dt.bfloat16

    nc = bacc.Bacc("TRN2", target_bir_lowering=False, debug=False)
    zt_d = nc.dram_tensor("z_t", [C, H, W], f32, kind="ExternalInput")
    z1_d = nc.dram_tensor("z_t1", [C, H, W], f32, kind="ExternalInput")
    out_d = nc.dram_tensor("out", [KK, H, W], f32, kind="ExternalOutput")

    def alt(i):
        return nc.sync if i % 2 == 0 else nc.scalar

    with tile.TileContext(nc) as tc:
        with tc.tile_pool(name="scrp", bufs=1, space="DRAM") as scrp:
            scr_t = [scrp.tile([W, SCR_W], bf16, tag=f"scr{i}", name=f"scr{i}")
                     for i in range(2)]

            # ================= stripe phase =================
            with (
                tc.tile_pool(name="persist", bufs=1) as pp,
                tc.tile_pool(name="ld", bufs=2) as ldp,
                tc.tile_pool(name="xbp", bufs=2) as xbp,
                tc.tile_pool(name="psp", bufs=2, space="PSUM") as psp,
            ):
                ZT = [pp.tile([128, H * W], bf16, tag=f"zt{k}", name=f"zt{k}")
                      for k in range(2)]
                Z1P = [pp.tile([128, HP * WP], bf16, tag=f"z1p{k}", name=f"z1p{k}")
                       for k in range(2)]
                for k in range(2):
                    nc.vector.memset(Z1P[k][:, :], 0.0)

                def emit_loads(s):
                    """Load 32-row slab s of z_t1 and z_t (both c-chunks)."""
                    for k in range(2):
                        z1u = ldp.tile([128, 32 * W], bf16, tag="z1u", name="z1u")
                        src = z1_d.ap()[k * 128:(k + 1) * 128, s * 32:(s + 1) * 32, :]
                        nc.gpsimd.dma_start(
                            z1u.rearrange("c (h w) -> c h w", h=32), src)
                        dst = Z1P[k].rearrange("c (h w) -> c h w", h=HP)[
                            :, RAD + s * 32: RAD + (s + 1) * 32, RAD: RAD + W]
                        nc.vector.tensor_copy(dst, z1u.rearrange("c (h w) -> c h w", h=32))
                    for k in range(2):
                        ztu = ldp.tile([128, 32 * W], bf16, tag="ztu", name="ztu")
                        src = zt_d.ap()[k * 128:(k + 1) * 128, s * 32:(s + 1) * 32, :]
                        nc.gpsimd.dma_start(
                            ztu.rearrange("c (h w) -> c h w", h=32), src)
                        # reorder to block-major with fused 1/16 scale
                        for sl in range(4):
                            si_g = s * 4 + sl
                            srcv = ztu.rearrange(
                                "c (h wb dw) -> c wb h dw", h=32, wb=NWB)[
                                :, :, sl * SA:(sl + 1) * SA, :]
                            dstv = ZT[k][:, si_g * 1024:(si_g + 1) * 1024].rearrange(
                                "c (wb dh dw) -> c wb dh dw", wb=NWB, dh=SA)
                            nc.vector.tensor_scalar_mul(dstv, srcv, 1.0 / 16.0)

                def emit_stripe(si):
                    hh, sil = divmod(si, 8)
                    h0 = si * SA
                    xb = xbp.tile([128, NWB * PQ], bf16, tag="xb", name="xb")
                    for wb in range(NWB):
                        w0 = wb * SB
                        ps = [psp.tile([128, 10 * WINQ], f32, tag=f"ps{i}", name=f"ps{i}")
                              for i in range(2)]
                        for k in range(2):
                            blk = si * NWB + wb
                            lhsT = ZT[k][:, blk * 128:(blk + 1) * 128]
                            for half in range(2):
                                rhs = Z1P[k].rearrange("c (h w) -> c h w", h=HP)[
                                    :, h0 + 10 * half: h0 + 10 * (half + 1),
                                    w0:w0 + WINQ]
                                nc.tensor.matmul(ps[half][:, :], lhsT, rhs,
                                                 start=(k == 0), stop=(k == 1))
                        for half in range(2):
                            dst = xb[:, wb * PQ + half * 10 * QP:
                                     wb * PQ + (half + 1) * 10 * QP].rearrange(
                                "p (r q) -> p r q", q=QP)[:, :, :WINQ]
                            srcp = ps[half].rearrange("p (r q) -> p r q", q=WINQ)
                            if wb % 2 == 0:
                                nc.scalar.copy(dst, srcp)
                            else:
                                nc.vector.tensor_copy(dst, srcp)

                    # sheared scratch write: one DMA per wb, 1280B runs.
                    for wb in range(NWB):
                        off = sil * SA * SCR_H + wb * SB * SCR_W + 7 * QP
                        dst = bass.AP(scr_t[hh].tensor, off,
                                      [[SCR_H - QP, SA], [SCR_W, SB], [1, PQ]])
                        alt(wb).dma_start(dst, xb[:, wb * PQ:(wb + 1) * PQ])

                emit_loads(0)
                emit_stripe(0)
                emit_stripe(1)
                emit_loads(1)
                for si in range(2, 6):
                    emit_stripe(si)
                emit_loads(2)
                for si in range(6, 10):
                    emit_stripe(si)
                emit_loads(3)
                for si in range(10, 16):
                    emit_stripe(si)

            # ================= tap phase =================
            with (
                tc.tile_pool(name="bnd", bufs=2) as bndp,
                tc.tile_pool(name="o5p", bufs=1) as o5p,
            ):
                o5 = [o5p.tile([HH, KS * W], f32, tag=f"o5_{di}", name=f"o5_{di}")
                      for di in range(KS)]
                for hh in range(2):
                    for wb in range(NWB):
                        band = bndp.tile([HH, SB * BAND], bf16, tag="band", name="band")
                        src = bass.AP(scr_t[hh].tensor,
                                      wb * SB * SCR_W + 7 * QP,
                                      [[SCR_H, HH], [SCR_W, SB], [1, BAND]])
                        alt(wb).dma_start(
                            band.rearrange("p (w e) -> p w e", e=BAND), src)
                        # de-shear: o5[di][p, dj*128 + wb*16 + wh]
                        #   = band[p, wh*416 + di*32 + wh + dj]
                        bap = band[:, :]
                        for di in range(KS):
                            diag = bass.AP(bap.tensor, bap.offset + di * QP,
                                           [list(bap.ap[0]),
                                            [BAND + 1, SB], [1, KS]])
                            dst = o5[di][:, :].rearrange(
                                "p (dj w) -> p dj w", dj=KS)[
                                :, :, wb * SB:(wb + 1) * SB].transpose([0, 2, 1])
                            if (wb + di) % 2 == 0:
                                nc.vector.tensor_copy(dst, diag)
                            else:
                                nc.scalar.copy(dst, diag)
                    for di in range(KS):
                        dstw = bass.AP(out_d, di * KS * H * W + hh * HH * W,
                                       [[W, HH], [H * W, KS], [1, W]])
                        alt(di).dma_start(
                            dstw, o5[di][:, :].rearrange("p (dj w) -> p dj w", dj=KS))

    nc.compile()
    return nc


def _get_nc():
    if "nc" not in _cache:
        _cache["nc"] = _build()
    return _cache["nc"]


def kernel(z_t: np.ndarray, z_t1: np.ndarray) -> np.ndarray:
    from concourse.bass_utils import run_bass_kernel_spmd

    nc = _get_nc()
    z_t = np.ascontiguousarray(z_t, dtype=np.float32)
    z_t1 = np.ascontiguousarray(z_t1, dtype=np.float32)
    B = z_t.shape[0]
    in_maps = [{"z_t": z_t[i], "z_t1": z_t1[i]} for i in range(B)]
    res = run_bass_kernel_spmd(nc, in_maps, core_ids=list(range(B)))
    return np.stack([res.results[i]["out"] for i in range(B)], axis=0)


# revision 10
# speedup vs baseline: 3.8005x; 1.0216x over previous
"""LocalCorrelation (13x13 cost volume) Trainium2 kernel.

Full inputs z_t, z_t1: [8, 256, 128, 128] f32 -> out [8, 169, 128, 128] f32.
out[b, 13*di+dj, h, w] = sum_c z_t[b,c,h,w] * pad(z_t1)[b,c,h+di,w+dj] / 16

Sharding: data-parallel over batch, 1 batch element per NeuronCore (8 cores).

Per-core algorithm (SPMD, identical program):
  - Slab-staged input load (f32->bf16 SWDGE cast) interleaved with compute;
    the 1/sqrt(C) scale is fused into the z_t reorder copy.
  - Per 8x16 output-pixel block: TensorE "block gram" matmuls, stationary =
    z_t block [c,128 pixels], streaming = padded z_t1 20x28 window -> PSUM
    f32 (accumulated over 2 c-chunks of 128).
  - PSUM -> SBUF bf16 with 32-elem window-row pitch.
  - Sheared scratch write: DRAM addr = w*S_W + h*S_H + (p - dh + 7)*32 + q.
    The per-pixel vertical shear is absorbed into the write AP's dh
    partition stride (S_H - 32), so window row p = dh + di for every pixel
    lands at slot di+7.
  - Band gather: per (h-half, wb) one 3-dim DMA reads slots [7,20) of every
    pixel -- 832B contiguous runs, 16 instructions, all 13 tap rows at once.
  - DVE de-shear: slot pick + horizontal diagonal q = (w mod 16) + dj via
    strided copy (+ f32 cast) into per-di output tiles.
  - Output write: one DMA per (h-half, di), 512B runs.
"""

import numpy as np

C = 256
H = W = 128
KS = 13
KK = 169
RAD = 6
HP = WP = 140  # padded spatial
SA = 8  # block rows (stripe height)
SB = 16  # block cols
NWB = W // SB  # 8 w-blocks per stripe
NST = H // SA  # 16 stripes
WINP = SA + 2 * RAD  # 20 streamed rows per window
WINQ = SB + 2 * RAD  # 28 streamed cols per window
QP = 32  # padded window-row pitch in xb/scratch
PQ = WINP * QP  # 640 elems per pixel in xb
SCR_H = 27 * QP  # 864: scratch stride per h (27 slots: p-dh+7 in [0,27))
SCR_W = 64 * SCR_H  # 55296: scratch stride per w (per h-half)
HH = H // 2  # 64 rows per h-half
BAND = KS * QP  # 416: gathered band elems per pixel (slots 7..19)

_cache = {}


def _build():
    import concourse.bass as bass
    import concourse.mybir as mybir
    import concourse.tile as tile
    from concourse import bacc

    f32 = mybir.dt.float32
    bf16 = mybir.dt.bfloat16

    nc = bacc.Bacc("TRN2", target_bir_lowering=False, debug=False)
    zt_d = nc.dram_tensor("z_t", [C, H, W], f32, kind="ExternalInput")
    z1_d = nc.dram_tensor("z_t1", [C, H, W], f32, kind="ExternalInput")
    out_d = nc.dram_tensor("out", [KK, H, W], f32, kind="ExternalOutput")

    def alt(i):
        return nc.sync if i % 2 == 0 else nc.scalar

    with tile.TileContext(nc) as tc:
        with tc.tile_pool(name="scrp", bufs=1, space="DRAM") as scrp:
            scr_t = [scrp.tile([W, SCR_W], bf16, tag=f"scr{i}", name=f"scr{i}")
                     for i in range(2)]

            # ================= stripe phase =================
            with (
                tc.tile_pool(name="persist", bufs=1) as pp,
                tc.tile_pool(name="ld", bufs=2) as ldp,
                tc.tile_pool(name="xbp", bufs=2) as xbp,
                tc.tile_pool(name="psp", bufs=2, space="PSUM") as psp,
            ):
                ZT = [pp.tile([128, H * W], bf16, tag=f"zt{k}", name=f"zt{k}")
                      for k in range(2)]
                Z1P = [pp.tile([128, HP * WP], bf16, tag=f"z1p{k}", name=f"z1p{k}")
                       for k in range(2)]
                for k in range(2):
                    nc.vector.memset(Z1P[k][:, :], 0.0)

                def emit_loads(s):
                    """Load 32-row slab s of z_t1 and z_t (both c-chunks)."""
                    for k in range(2):
                        z1u = ldp.tile([128, 32 * W], bf16, tag="z1u", name="z1u")
                        src = z1_d.ap()[k * 128:(k + 1) * 128, s * 32:(s + 1) * 32, :]
                        nc.gpsimd.dma_start(
                            z1u.rearrange("c (h w) -> c h w", h=32), src)
                        dst = Z1P[k].rearrange("c (h w) -> c h w", h=HP)[
                            :, RAD + s * 32: RAD + (s + 1) * 32, RAD: RAD + W]
                        nc.vector.tensor_copy(dst, z1u.rearrange("c (h w) -> c h w", h=32))
                    for k in range(2):
                        ztu = ldp.tile([128, 32 * W], bf16, tag="ztu", name="ztu")
                        src = zt_d.ap()[k * 128:(k + 1) * 128, s * 32:(s + 1) * 32, :]
                        nc.gpsimd.dma_start(
                            ztu.rearrange("c (h w) -> c h w", h=32), src)
                        # reorder to block-major with fused 1/16 scale
                        for sl in range(4):
                            si_g = s * 4 + sl
                            srcv = ztu.rearrange(
                                "c (h wb dw) -> c wb h dw", h=32, wb=NWB)[
                                :, :, sl * SA:(sl + 1) * SA, :]
                            dstv = ZT[k][:, si_g * 1024:(si_g + 1) * 1024].rearrange(
                                "c (wb dh dw) -> c wb dh dw", wb=NWB, dh=SA)
                            nc.vector.tensor_scalar_mul(dstv, srcv, 1.0 / 16.0)

                def emit_stripe(si):
                    hh, sil = divmod(si, 8)
                    h0 = si * SA
                    xb = xbp.tile([128, NWB * PQ], bf16, tag="xb", name="xb")
                    for wb in range(NWB):
                        w0 = wb * SB
                        ps = [psp.tile([128, 10 * WINQ], f32, tag=f"ps{i}", name=f"ps{i}")
                              for i in range(2)]
                        for k in range(2):
                            blk = si * NWB + wb
                            lhsT = ZT[k][:, blk * 128:(blk + 1) * 128]
                            for half in range(2):
                                rhs = Z1P[k].rearrange("c (h w) -> c h w", h=HP)[
                                    :, h0 + 10 * half: h0 + 10 * (half + 1),
                                    w0:w0 + WINQ]
                                nc.tensor.matmul(ps[half][:, :], lhsT, rhs,
                                                 start=(k == 0), stop=(k == 1))
                        for half in range(2):
                            dst = xb[:, wb * PQ + half * 10 * QP:
                                     wb * PQ + (half + 1) * 10 * QP].rearrange(
                                "p (r q) -> p r q", q=QP)[:, :, :WINQ]
                            srcp = ps[half].rearrange("p (r q) -> p r q", q=WINQ)
                            if wb % 2 == 0:
                                nc.scalar.copy(dst, srcp)
                            else:
                                nc.vector.tensor_copy(dst, srcp)

                    # sheared scratch write: one DMA per wb, 1280B runs.
                    for wb in range(NWB):
                        off = sil * SA * SCR_H + wb * SB * SCR_W + 7 * QP
                        dst = bass.AP(scr_t[hh].tensor, off,
                                      [[SCR_H - QP, SA], [SCR_W, SB], [1, PQ]])
                        alt(wb).dma_start(dst, xb[:, wb * PQ:(wb + 1) * PQ])

                emit_loads(0)
                emit_stripe(0)
                emit_stripe(1)
                emit_loads(1)
                for si in range(2, 6):
                    emit_stripe(si)
                emit_loads(2)
                for si in range(6, 10):
                    emit_stripe(si)
                emit_loads(3)
                for si in range(10, 16):
                    emit_stripe(si)

            # ================= tap phase =================
            with (
                tc.tile_pool(name="bnd", bufs=2) as bndp,
                tc.tile_pool(name="o5p", bufs=1) as o5p,
            ):
                # o5[di] partitions = full h (both halves) so output writes
                # spread across all 16 DMA engines.
                o5 = [o5p.tile([H, KS * W], f32, tag=f"o5_{di}", name=f"o5_{di}")
                      for di in range(KS)]
                for hh in range(2):
                    for wb in range(NWB):
                        # alternate partition halves so consecutive gathers
                        # use disjoint SDMA engine sets (partition swizzle).
                        par = (hh * NWB + wb) % 2
                        bt = bndp.tile([128, SB * BAND], bf16, tag="band", name="band")
                        band = bt[par * HH:(par + 1) * HH, :]
                        src = bass.AP(scr_t[hh].tensor,
                                      wb * SB * SCR_W + 7 * QP,
                                      [[SCR_H, HH], [SCR_W, SB], [1, BAND]])
                        alt(wb).dma_start(
                            band.rearrange("p (w e) -> p w e", e=BAND), src)
                        # de-shear: o5[di][hh*64+p, dj*128 + wb*16 + wh]
                        #   = band[p, wh*416 + di*32 + wh + dj]
                        bap = band
                        for di in range(KS):
                            diag = bass.AP(bap.tensor, bap.offset + di * QP,
                                           [list(bap.ap[0]),
                                            [BAND + 1, SB], [1, KS]])
                            dst = o5[di][hh * HH:(hh + 1) * HH, :].rearrange(
                                "p (dj w) -> p dj w", dj=KS)[
                                :, :, wb * SB:(wb + 1) * SB].transpose([0, 2, 1])
                            if (wb + di) % 2 == 0:
                                nc.vector.tensor_copy(dst, diag)
                            else:
                                nc.scalar.copy(dst, diag)
                for di in range(KS):
                    dstw = bass.AP(out_d, di * KS * H * W,
                                   [[W, H], [H * W, KS], [1, W]])
                    alt(di).dma_start(
                        dstw, o5[di][:, :].rearrange("p (dj w) -> p dj w", dj=KS))

    nc.compile()
    return nc


def _get_nc():
    if "nc" not in _cache:
        _cache["nc"] = _build()
    return _cache["nc"]


def kernel(z_t: np.ndarray, z_t1: np.ndarray) -> np.ndarray:
    from concourse.bass_utils import run_bass_kernel_spmd

    nc = _get_nc()
    z_t = np.ascontiguousarray(z_t, dtype=np.float32)
    z_t1 = np.ascontiguousarray(z_t1, dtype=np.float32)
    B = z_t.shape[0]
    in_maps = [{"z_t": z_t[i], "z_t1": z_t1[i]} for i in range(B)]
    res = run_bass_kernel_spmd(nc, in_maps, core_ids=list(range(B)))
    return np.stack([res.results[i]["out"] for i in range(B)], axis=0)
